# revision 67
# baseline (speedup 1.0000x reference)
"""Causal depthwise conv1d with learnable hidden-state prefix, on 8 TRN2 cores.

Reference computation (per batch b, channel d):
    xp = concat([init_state[d, :3], x[b, d, :]])          # [L+3] = [4099]
    out[b, d, t] = bias[d] + sum_{j=0..3} w[d, j] * xp[t+j]   for t in [0, 4099)
    (xp index beyond 4098 contributes 0)

Sharding: channel dim D=4096 split 8 ways (512 channels/core), zero
communication. Each core processes rows (b, d_local) = 4*512 = 2048 rows of
length 4096 -> 16 SBUF tiles of [128 rows, full row].

The kernel is DMA-bound: fp32 input (16.4KB/row) + bf16 output (8.2KB/row)
= ~140us at the 360GB/s descriptor-model roofline. Storing the output as
bf16 only rounds the final fp32 result (<= 2^-9 relative, inside the 2e-2
tolerance); the host upcasts back. All arithmetic stays fp32 (fp32r is an
11-bit-mantissa format - measured 2^-12 operand rounding violates the
near-zero-output error budget, so it is not used).

To fit every engine under the DMA roofline the 4-tap conv is spread over
all five engines per tile:
  - PE: first `pe_chunks` 512-col chunks as 4 accumulating diagonal-weight
    fp32 matmuls per PSUM bank; ACT evacuates + bias -> bf16.
  - remaining cols via a MAC chain in sub-pieces (pipelined): ACT computes
    A = w0*x0 + bias and B = w2*x2 (fp32 tmps), Pool merges A += B (plain
    TensorTensor Add - the only elementwise shape Pool's ISA accepts; STT
    with a per-partition scalar pointer is PE/ACT/DVE-only), DVE runs
    A += w1*x1 and the final out_bf16 = w3*x3 + A.  3 zero cols are padded
    after x so every tap runs full width (xp reads past the end get 0).
Out-DMA pieces are deferred a few tiles (banking transfer work for the
drain phase) and ride rings cross-engine from their producers: chain
pieces on the ACT HWDGE ring (DVE produces them), PE pieces on Pool
SWDGE, and everything on the idle SP ring during the drain. The last two
tiles run a third PE chunk (the PE is otherwise done) to shorten the
final chain latency.
"""

import numpy as np

B, D, L = 4, 4096, 4096
KTAPS = 4
K = KTAPS - 1          # 3: state length
LOUT = L + K           # 4099
NCORES = 8
DSH = D // NCORES      # 512 channels per core
ROWS = B * DSH         # 2048 rows per core
P = 128                # SBUF partitions
NTILES = ROWS // P     # 16
G = DSH // P           # 4 channel groups per core
MMCOLS = 512           # one PSUM bank of fp32 per matmul
WIN = 1 + K + L + K    # in_t cols: align pad, state, x, zero pad = 4103

_CACHE = {}


def _build_program(in_bufs=6, out_bufs=5, pe_chunks=(2,) * NTILES,
                   in_pieces=2, subs_m=3, subs_v=1, m_cols=3075,
                   chain_outs=1, out_lag=4, pe_tail=3, pe_tail_n=2,
                   ta_bufs=3, tb_bufs=2):
    import concourse.bacc as bacc
    import concourse.mybir as mybir
    from concourse.tile import TileContext

    f32 = mybir.dt.float32
    bf16 = mybir.dt.bfloat16
    nc = bacc.Bacc("TRN2", target_bir_lowering=False, debug=False)

    xs = nc.dram_tensor("xs", [ROWS, L], f32, kind="ExternalInput").ap()
    # single packed param tensor -> single DMA -> single sync wait downstream.
    # layout per partition p: cols [g*4+j]=w[g*128+p, j] for g<4,j<4 (0..16),
    # col 16+g = bias[g*128+p], col 20+g*3+k = init_state[g*128+p, k]
    prm_d = nc.dram_tensor("prm", [P, 36], f32, kind="ExternalInput").ap()
    eye_d = nc.dram_tensor("eye", [P, P], f32, kind="ExternalInput").ap()
    out_d = nc.dram_tensor("out", [ROWS, LOUT], bf16, kind="ExternalOutput").ap()

    if pe_tail != 2:
        pe_chunks = (tuple(pe_chunks[:-pe_tail_n])
                     + (pe_tail,) * pe_tail_n)

    with TileContext(nc) as tc:
        with (
            tc.tile_pool(name="consts", bufs=1) as cpool,
            tc.tile_pool(name="xin", bufs=in_bufs) as in_pool,
            tc.tile_pool(name="yout", bufs=out_bufs) as out_pool,
            tc.tile_pool(name="tmpa", bufs=ta_bufs) as ta_pool,
            tc.tile_pool(name="tmpb", bufs=tb_bufs) as tb_pool,
            tc.tile_pool(name="psum", bufs=8, space="PSUM") as ps_pool,
        ):
            lpp = L // in_pieces

            def load_x(t):
                # in_t: col 0 pad (16B align), state [1:4), x [4:4100),
                # zero pad [4100:4103). x lands in pieces so chunks start
                # before the whole row arrives.
                in_t = in_pool.tile([P, WIN], f32)
                for p in range(in_pieces):
                    nc.sync.dma_start(
                        out=in_t[:, 1 + K + p * lpp:1 + K + (p + 1) * lpp],
                        in_=xs[t * P:(t + 1) * P, p * lpp:(p + 1) * lpp])
                return in_t

            # tile 0's x DMA first: it heads the SP/HWDGE pipeline, so its
            # transfer (the critical first 2.9us) isn't queued behind the
            # param loads, which have tiles of slack before first use.
            in_t0 = load_x(0)

            prm = cpool.tile([P, 36], f32)
            nc.sync.dma_start(out=prm, in_=prm_d)
            w_sb = prm[:, 0:G * KTAPS]
            b_sb = prm[:, 16:16 + G]
            s_sb = prm[:, 20:20 + G * K]
            zero = prm[:, 32:33]  # col 32: host leaves it zero (20:32 is state)

            # per-(group, tap) diagonal weight matrices for the PE path
            eye = cpool.tile([P, P], f32)
            nc.sync.dma_start(out=eye, in_=eye_d)
            dg = {}
            for g in range(G):
                for j in range(KTAPS):
                    d = cpool.tile([P, P], f32, tag=f"diag{g}_{j}")
                    nc.vector.tensor_scalar_mul(
                        out=d, in0=eye,
                        scalar1=w_sb[:, g * KTAPS + j:g * KTAPS + j + 1])
                    dg[(g, j)] = d

            lagged = []  # [tile, thunk] out-DMAs deferred by out_lag tiles

            for t in range(NTILES):
                g = t % G  # channel group (tile order: batch-major)
                rows = slice(t * P, (t + 1) * P)
                wj = [w_sb[:, g * KTAPS + j:g * KTAPS + j + 1]
                      for j in range(KTAPS)]

                in_t = in_t0 if t == 0 else load_x(t)
                nc.vector.tensor_scalar_mul(
                    out=in_t[:, 1:1 + K], in0=s_sb[:, g * K:(g + 1) * K],
                    scalar1=1.0)
                nc.vector.memset(in_t[:, 1 + K + L:], 0.0)

                out_t = out_pool.tile([P, LOUT], bf16)

                # flush out-DMAs deferred from >= out_lag tiles ago (all of
                # them on the last tile): their data is long done, so these
                # never park an engine SEQ mid-pipeline. The deferral banks
                # transfer work for the drain phase, when the remaining
                # compute can no longer saturate the DMA engines.
                flush_upto = t - out_lag
                lagged = [(tt, fn) for tt, fn in lagged
                          if not (tt <= flush_upto and (fn() or True))]

                # --- PE region: cols [0, ncols) ---
                # Evacs are emitted AFTER the chain taps (below): ACT's
                # in-order stream must not fence the PE-independent chain
                # work on the PE's PSUM completions.
                ncols = pe_chunks[t] * MMCOLS
                pss = []
                for c in range(pe_chunks[t]):
                    ps = ps_pool.tile([P, MMCOLS], f32)
                    base = 1 + c * MMCOLS
                    for j in range(KTAPS):
                        nc.tensor.matmul(
                            ps, dg[(g, j)],
                            in_t[:, base + j:base + j + MMCOLS],
                            start=(j == 0), stop=(j == KTAPS - 1))
                    pss.append(ps)

                # --- chain region: cols [ncols, LOUT) ---
                # merge flavor on [ncols, mend): ACT computes both chain
                # starts, Pool TT-adds them, DVE finishes (2 STTs).
                # DVE-serial flavor on the rest: ACT start + 3 DVE STTs.
                # Sized so ACT/DVE/Pool each stay under the DMA cadence.
                top = ncols
                mend = min(m_cols + ncols, LOUT)  # m boundary
                mtop = mend - top                 # m cols
                ub = [top + (mtop * s) // subs_m for s in range(subs_m + 1)]
                ub += [mend + ((LOUT - mend) * s) // subs_v
                       for s in range(1, subs_v + 1)]
                pairs = [(ub[s], ub[s + 1]) for s in range(len(ub) - 1)
                         if ub[s + 1] > ub[s]]
                for s, (lo, hi) in enumerate(pairs):
                    merge = hi <= mend
                    i0, i1 = 0, hi - lo
                    ta = ta_pool.tile([P, hi - lo], f32, tag=f"ta{s}")
                    if merge:
                        tb = tb_pool.tile([P, hi - lo], f32, tag=f"tb{s}")
                    nc.scalar.activation(
                        ta[:, i0:i1], in_t[:, 1 + lo:1 + hi],
                        mybir.ActivationFunctionType.Identity,
                        bias=b_sb[:, g:g + 1], scale=wj[0])
                    if merge:
                        nc.scalar.activation(
                            tb[:, i0:i1], in_t[:, 3 + lo:3 + hi],
                            mybir.ActivationFunctionType.Identity,
                            bias=zero, scale=wj[2])
                        nc.gpsimd.tensor_tensor(
                            out=ta[:, i0:i1], in0=ta[:, i0:i1],
                            in1=tb[:, i0:i1], op=mybir.AluOpType.add)
                    else:
                        nc.vector.scalar_tensor_tensor(
                            out=ta[:, i0:i1],
                            in0=in_t[:, 3 + lo:3 + hi],
                            scalar=wj[2],
                            in1=ta[:, i0:i1],
                            op0=mybir.AluOpType.mult,
                            op1=mybir.AluOpType.add,
                        )
                    nc.vector.scalar_tensor_tensor(
                        out=ta[:, i0:i1],
                        in0=in_t[:, 2 + lo:2 + hi],
                        scalar=wj[1],
                        in1=ta[:, i0:i1],
                        op0=mybir.AluOpType.mult,
                        op1=mybir.AluOpType.add,
                    )
                    nc.vector.scalar_tensor_tensor(
                        out=out_t[:, lo:hi],
                        in0=in_t[:, 4 + lo:4 + hi],
                        scalar=wj[3],
                        in1=ta[:, i0:i1],
                        op0=mybir.AluOpType.mult,
                        op1=mybir.AluOpType.add,
                    )

                for c, ps in enumerate(pss):
                    nc.scalar.activation(
                        out_t[:, c * MMCOLS:(c + 1) * MMCOLS],
                        ps, mybir.ActivationFunctionType.Identity,
                        bias=b_sb[:, g:g + 1], scale=1.0)

                # --- out DMAs ---
                # PE piece on the SP ring (producer: ACT evac; lagging keeps
                # its wait pre-satisfied so it never head-of-line-blocks the
                # in stream); chain pieces on the ACT ring (producer: DVE
                # final STT) - cross-engine in both cases, so a real
                # semaphore guards the HW DMA against the producer pipeline.
                # All are lagged out_lag tiles: the banked transfers keep the
                # DMA engines fed during the drain phase, when the remaining
                # compute alone can't saturate them. The last tile ships one
                # piece per chain sub so the final transfer is small.
                def mk(ring, lo, hi, out_t=out_t, rows=rows):
                    def fire(last=False):
                        r = nc.sync if last else ring
                        r.dma_start(out=out_d[rows, lo:hi],
                                    in_=out_t[:, lo:hi])
                    return fire
                if t == NTILES - 1:
                    # drain: banked pieces ride the now-idle SP ring in tile
                    # order (oldest chains finish first), this tile's pieces
                    # follow per-sub on their usual rings. All engine compute
                    # is already dispatched, so SEQ parking is harmless.
                    for tt, fn in lagged:
                        fn(True)
                    lagged = []
                    mk(nc.sync, 0, ncols)()
                    for lo, hi in pairs:
                        mk(nc.scalar, lo, hi)()
                else:
                    thunks = [mk(nc.gpsimd, 0, ncols)]
                    cb = [ncols + ((LOUT - ncols) * s) // chain_outs
                          for s in range(chain_outs + 1)]
                    thunks += [mk(nc.scalar, cb[s], cb[s + 1])
                               for s in range(chain_outs)]
                    lagged += [(t, fn) for fn in thunks]

    nc.compile()
    return nc


def kernel(x, weight, bias, init_state):
    from concourse.bass_utils import run_bass_kernel_spmd

    assert x.shape == (B, D, L) and x.dtype == np.float32
    wl = np.ascontiguousarray(weight[:, 0, :], dtype=np.float32)      # [D, 4]
    bias = np.ascontiguousarray(bias, dtype=np.float32)               # [D]
    st = np.ascontiguousarray(init_state, dtype=np.float32)           # [D, 3]

    if "nc" not in _CACHE:
        _CACHE["nc"] = _build_program()
    nc = _CACHE["nc"]

    in_maps = []
    for c in range(NCORES):
        lo, hi = c * DSH, (c + 1) * DSH
        xs = np.ascontiguousarray(x[:, lo:hi, :]).reshape(ROWS, L)
        wc = wl[lo:hi]                                                # [512, 4]
        prm = np.zeros((P, 36), np.float32)
        prm[:, 0:G * KTAPS] = (
            wc.reshape(G, P, KTAPS).transpose(1, 0, 2).reshape(P, G * KTAPS))
        prm[:, 16:16 + G] = bias[lo:hi].reshape(G, P).T
        prm[:, 20:20 + G * K] = (
            st[lo:hi].reshape(G, P, K).transpose(1, 0, 2).reshape(P, G * K))
        in_maps.append({"xs": xs, "prm": prm,
                        "eye": np.eye(P, dtype=np.float32)})

    res = run_bass_kernel_spmd(nc, in_maps, core_ids=list(range(NCORES)))
    shards = [np.asarray(r["out"]).astype(np.float32).reshape(B, DSH, LOUT)
              for r in res.results]
    return np.ascontiguousarray(np.concatenate(shards, axis=1))


# revision 70
# speedup vs baseline: 1.0072x; 1.0072x over previous
"""Causal depthwise conv1d with learnable hidden-state prefix, on 8 TRN2 cores.

Reference computation (per batch b, channel d):
    xp = concat([init_state[d, :3], x[b, d, :]])          # [L+3] = [4099]
    out[b, d, t] = bias[d] + sum_{j=0..3} w[d, j] * xp[t+j]   for t in [0, 4099)
    (xp index beyond 4098 contributes 0)

Sharding: channel dim D=4096 split 8 ways (512 channels/core), zero
communication. Each core processes rows (b, d_local) = 4*512 = 2048 rows of
length 4096 -> 16 SBUF tiles of [128 rows, full row].

The kernel is DMA-bound: fp32 input (16.4KB/row) + bf16 output (8.2KB/row)
= ~140us at the 360GB/s descriptor-model roofline. Storing the output as
bf16 only rounds the final fp32 result (<= 2^-9 relative, inside the 2e-2
tolerance); the host upcasts back. All arithmetic stays fp32 (fp32r is an
11-bit-mantissa format - measured 2^-12 operand rounding violates the
near-zero-output error budget, so it is not used).

To fit every engine under the DMA roofline the 4-tap conv is spread over
all five engines per tile:
  - PE: first `pe_chunks` 512-col chunks as 4 accumulating diagonal-weight
    fp32 matmuls per PSUM bank; ACT evacuates + bias -> bf16.
  - remaining cols via a MAC chain in sub-pieces (pipelined): ACT computes
    A = w0*x0 + bias and B = w2*x2 (fp32 tmps), Pool merges A += B (plain
    TensorTensor Add - the only elementwise shape Pool's ISA accepts; STT
    with a per-partition scalar pointer is PE/ACT/DVE-only), DVE runs
    A += w1*x1 and the final out_bf16 = w3*x3 + A.  3 zero cols are padded
    after x so every tap runs full width (xp reads past the end get 0).
Out-DMA pieces are deferred a few tiles (banking transfer work for the
drain phase) and ride rings cross-engine from their producers: chain
pieces on the ACT HWDGE ring (DVE produces them), PE pieces on Pool
SWDGE, and everything on the idle SP ring during the drain. The last two
tiles run a third PE chunk (the PE is otherwise done) to shorten the
final chain latency.
"""

import numpy as np

B, D, L = 4, 4096, 4096
KTAPS = 4
K = KTAPS - 1          # 3: state length
LOUT = L + K           # 4099
NCORES = 8
DSH = D // NCORES      # 512 channels per core
ROWS = B * DSH         # 2048 rows per core
P = 128                # SBUF partitions
NTILES = ROWS // P     # 16
G = DSH // P           # 4 channel groups per core
MMCOLS = 512           # one PSUM bank of fp32 per matmul
WIN = 1 + K + L + K    # in_t cols: align pad, state, x, zero pad = 4103

_CACHE = {}


def _build_program(in_bufs=6, out_bufs=5, pe_chunks=(2,) * NTILES,
                   in_pieces=2, subs_m=3, subs_v=1, m_cols=3075,
                   chain_outs=1, out_lag=4, pe_tail=3, pe_tail_n=2,
                   ta_bufs=3, tb_bufs=2, pool_final=1):
    import concourse.bacc as bacc
    import concourse.mybir as mybir
    from concourse.tile import TileContext

    f32 = mybir.dt.float32
    bf16 = mybir.dt.bfloat16
    nc = bacc.Bacc("TRN2", target_bir_lowering=False, debug=False)

    xs = nc.dram_tensor("xs", [ROWS, L], f32, kind="ExternalInput").ap()
    # single packed param tensor -> single DMA -> single sync wait downstream.
    # layout per partition p: cols [g*4+j]=w[g*128+p, j] for g<4,j<4 (0..16),
    # col 16+g = bias[g*128+p], col 20+g*3+k = init_state[g*128+p, k]
    prm_d = nc.dram_tensor("prm", [P, 36], f32, kind="ExternalInput").ap()
    eye_d = nc.dram_tensor("eye", [P, P], f32, kind="ExternalInput").ap()
    out_d = nc.dram_tensor("out", [ROWS, LOUT], bf16, kind="ExternalOutput").ap()

    if pe_tail != 2:
        pe_chunks = (tuple(pe_chunks[:-pe_tail_n])
                     + (pe_tail,) * pe_tail_n)

    with TileContext(nc) as tc:
        with (
            tc.tile_pool(name="consts", bufs=1) as cpool,
            tc.tile_pool(name="xin", bufs=in_bufs) as in_pool,
            tc.tile_pool(name="yout", bufs=out_bufs) as out_pool,
            tc.tile_pool(name="tmpa", bufs=ta_bufs) as ta_pool,
            tc.tile_pool(name="tmpb", bufs=tb_bufs) as tb_pool,
            tc.tile_pool(name="psum", bufs=8, space="PSUM") as ps_pool,
        ):
            lpp = L // in_pieces

            def load_x(t):
                # in_t: col 0 pad (16B align), state [1:4), x [4:4100),
                # zero pad [4100:4103). x lands in pieces so chunks start
                # before the whole row arrives.
                in_t = in_pool.tile([P, WIN], f32)
                for p in range(in_pieces):
                    nc.sync.dma_start(
                        out=in_t[:, 1 + K + p * lpp:1 + K + (p + 1) * lpp],
                        in_=xs[t * P:(t + 1) * P, p * lpp:(p + 1) * lpp])
                return in_t

            # tile 0's x DMA first: it heads the SP/HWDGE pipeline, so its
            # transfer (the critical first 2.9us) isn't queued behind the
            # param loads, which have tiles of slack before first use.
            in_t0 = load_x(0)

            prm = cpool.tile([P, 36], f32)
            nc.sync.dma_start(out=prm, in_=prm_d)
            w_sb = prm[:, 0:G * KTAPS]
            b_sb = prm[:, 16:16 + G]
            s_sb = prm[:, 20:20 + G * K]
            zero = prm[:, 32:33]  # col 32: host leaves it zero (20:32 is state)

            # per-(group, tap) diagonal weight matrices for the PE path
            eye = cpool.tile([P, P], f32)
            nc.sync.dma_start(out=eye, in_=eye_d)
            dg = {}
            for g in range(G):
                for j in range(KTAPS):
                    d = cpool.tile([P, P], f32, tag=f"diag{g}_{j}")
                    nc.vector.tensor_scalar_mul(
                        out=d, in0=eye,
                        scalar1=w_sb[:, g * KTAPS + j:g * KTAPS + j + 1])
                    dg[(g, j)] = d

            lagged = []  # [tile, thunk] out-DMAs deferred by out_lag tiles

            for t in range(NTILES):
                g = t % G  # channel group (tile order: batch-major)
                rows = slice(t * P, (t + 1) * P)
                wj = [w_sb[:, g * KTAPS + j:g * KTAPS + j + 1]
                      for j in range(KTAPS)]

                in_t = in_t0 if t == 0 else load_x(t)
                nc.vector.tensor_scalar_mul(
                    out=in_t[:, 1:1 + K], in0=s_sb[:, g * K:(g + 1) * K],
                    scalar1=1.0)
                nc.vector.memset(in_t[:, 1 + K + L:], 0.0)

                out_t = out_pool.tile([P, LOUT], bf16)

                # flush out-DMAs deferred from >= out_lag tiles ago (all of
                # them on the last tile): their data is long done, so these
                # never park an engine SEQ mid-pipeline. The deferral banks
                # transfer work for the drain phase, when the remaining
                # compute can no longer saturate the DMA engines.
                flush_upto = t - out_lag
                lagged = [(tt, fn) for tt, fn in lagged
                          if not (tt <= flush_upto and (fn() or True))]

                # --- PE region: cols [0, ncols) ---
                # Evacs are emitted AFTER the chain taps (below): ACT's
                # in-order stream must not fence the PE-independent chain
                # work on the PE's PSUM completions.
                ncols = pe_chunks[t] * MMCOLS
                pss = []
                for c in range(pe_chunks[t]):
                    ps = ps_pool.tile([P, MMCOLS], f32)
                    base = 1 + c * MMCOLS
                    for j in range(KTAPS):
                        nc.tensor.matmul(
                            ps, dg[(g, j)],
                            in_t[:, base + j:base + j + MMCOLS],
                            start=(j == 0), stop=(j == KTAPS - 1))
                    pss.append(ps)

                # --- chain region: cols [ncols, LOUT) ---
                # merge flavor on [ncols, mend): ACT computes both chain
                # starts, Pool TT-adds them, DVE finishes (2 STTs).
                # DVE-serial flavor on the rest: ACT start + 3 DVE STTs.
                # Sized so ACT/DVE/Pool each stay under the DMA cadence.
                top = ncols
                mend = min(m_cols + ncols, LOUT)  # m boundary
                mtop = mend - top                 # m cols
                ub = [top + (mtop * s) // subs_m for s in range(subs_m + 1)]
                ub += [mend + ((LOUT - mend) * s) // subs_v
                       for s in range(1, subs_v + 1)]
                pairs = [(ub[s], ub[s + 1]) for s in range(len(ub) - 1)
                         if ub[s + 1] > ub[s]]
                for s, (lo, hi) in enumerate(pairs):
                    merge = hi <= mend
                    i0, i1 = 0, hi - lo
                    ta = ta_pool.tile([P, hi - lo], f32, tag=f"ta{s}")
                    if merge:
                        tb = tb_pool.tile([P, hi - lo], f32, tag=f"tb{s}")
                    nc.scalar.activation(
                        ta[:, i0:i1], in_t[:, 1 + lo:1 + hi],
                        mybir.ActivationFunctionType.Identity,
                        bias=b_sb[:, g:g + 1], scale=wj[0])
                    if merge and pool_final:
                        # 3-stage variant: two independent ACT+DVE
                        # sub-chains, Pool merges straight into bf16 out.
                        nc.scalar.activation(
                            tb[:, i0:i1], in_t[:, 3 + lo:3 + hi],
                            mybir.ActivationFunctionType.Identity,
                            bias=zero, scale=wj[2])
                        nc.vector.scalar_tensor_tensor(
                            out=ta[:, i0:i1], in0=in_t[:, 2 + lo:2 + hi],
                            scalar=wj[1], in1=ta[:, i0:i1],
                            op0=mybir.AluOpType.mult,
                            op1=mybir.AluOpType.add)
                        nc.vector.scalar_tensor_tensor(
                            out=tb[:, i0:i1], in0=in_t[:, 4 + lo:4 + hi],
                            scalar=wj[3], in1=tb[:, i0:i1],
                            op0=mybir.AluOpType.mult,
                            op1=mybir.AluOpType.add)
                        nc.gpsimd.tensor_tensor(
                            out=out_t[:, lo:hi], in0=ta[:, i0:i1],
                            in1=tb[:, i0:i1], op=mybir.AluOpType.add)
                        continue
                    if merge:
                        nc.scalar.activation(
                            tb[:, i0:i1], in_t[:, 3 + lo:3 + hi],
                            mybir.ActivationFunctionType.Identity,
                            bias=zero, scale=wj[2])
                        nc.gpsimd.tensor_tensor(
                            out=ta[:, i0:i1], in0=ta[:, i0:i1],
                            in1=tb[:, i0:i1], op=mybir.AluOpType.add)
                    else:
                        nc.vector.scalar_tensor_tensor(
                            out=ta[:, i0:i1],
                            in0=in_t[:, 3 + lo:3 + hi],
                            scalar=wj[2],
                            in1=ta[:, i0:i1],
                            op0=mybir.AluOpType.mult,
                            op1=mybir.AluOpType.add,
                        )
                    nc.vector.scalar_tensor_tensor(
                        out=ta[:, i0:i1],
                        in0=in_t[:, 2 + lo:2 + hi],
                        scalar=wj[1],
                        in1=ta[:, i0:i1],
                        op0=mybir.AluOpType.mult,
                        op1=mybir.AluOpType.add,
                    )
                    nc.vector.scalar_tensor_tensor(
                        out=out_t[:, lo:hi],
                        in0=in_t[:, 4 + lo:4 + hi],
                        scalar=wj[3],
                        in1=ta[:, i0:i1],
                        op0=mybir.AluOpType.mult,
                        op1=mybir.AluOpType.add,
                    )

                for c, ps in enumerate(pss):
                    nc.scalar.activation(
                        out_t[:, c * MMCOLS:(c + 1) * MMCOLS],
                        ps, mybir.ActivationFunctionType.Identity,
                        bias=b_sb[:, g:g + 1], scale=1.0)

                # --- out DMAs ---
                # PE piece on the SP ring (producer: ACT evac; lagging keeps
                # its wait pre-satisfied so it never head-of-line-blocks the
                # in stream); chain pieces on the ACT ring (producer: DVE
                # final STT) - cross-engine in both cases, so a real
                # semaphore guards the HW DMA against the producer pipeline.
                # All are lagged out_lag tiles: the banked transfers keep the
                # DMA engines fed during the drain phase, when the remaining
                # compute alone can't saturate them. The last tile ships one
                # piece per chain sub so the final transfer is small.
                def mk(ring, lo, hi, out_t=out_t, rows=rows):
                    def fire(last=False):
                        r = nc.sync if last else ring
                        r.dma_start(out=out_d[rows, lo:hi],
                                    in_=out_t[:, lo:hi])
                    return fire
                if t == NTILES - 1:
                    # drain: banked pieces ride the now-idle SP ring in tile
                    # order (oldest chains finish first), this tile's pieces
                    # follow per-sub on their usual rings. All engine compute
                    # is already dispatched, so SEQ parking is harmless.
                    for tt, fn in lagged:
                        fn(True)
                    lagged = []
                    mk(nc.sync, 0, ncols)()
                    for lo, hi in pairs:
                        mk(nc.scalar, lo, hi)()
                else:
                    thunks = [mk(nc.gpsimd, 0, ncols)]
                    cb = [ncols + ((LOUT - ncols) * s) // chain_outs
                          for s in range(chain_outs + 1)]
                    thunks += [mk(nc.scalar, cb[s], cb[s + 1])
                               for s in range(chain_outs)]
                    lagged += [(t, fn) for fn in thunks]

    nc.compile()
    return nc


def kernel(x, weight, bias, init_state):
    from concourse.bass_utils import run_bass_kernel_spmd

    assert x.shape == (B, D, L) and x.dtype == np.float32
    wl = np.ascontiguousarray(weight[:, 0, :], dtype=np.float32)      # [D, 4]
    bias = np.ascontiguousarray(bias, dtype=np.float32)               # [D]
    st = np.ascontiguousarray(init_state, dtype=np.float32)           # [D, 3]

    if "nc" not in _CACHE:
        _CACHE["nc"] = _build_program()
    nc = _CACHE["nc"]

    in_maps = []
    for c in range(NCORES):
        lo, hi = c * DSH, (c + 1) * DSH
        xs = np.ascontiguousarray(x[:, lo:hi, :]).reshape(ROWS, L)
        wc = wl[lo:hi]                                                # [512, 4]
        prm = np.zeros((P, 36), np.float32)
        prm[:, 0:G * KTAPS] = (
            wc.reshape(G, P, KTAPS).transpose(1, 0, 2).reshape(P, G * KTAPS))
        prm[:, 16:16 + G] = bias[lo:hi].reshape(G, P).T
        prm[:, 20:20 + G * K] = (
            st[lo:hi].reshape(G, P, K).transpose(1, 0, 2).reshape(P, G * K))
        in_maps.append({"xs": xs, "prm": prm,
                        "eye": np.eye(P, dtype=np.float32)})

    res = run_bass_kernel_spmd(nc, in_maps, core_ids=list(range(NCORES)))
    shards = [np.asarray(r["out"]).astype(np.float32).reshape(B, DSH, LOUT)
              for r in res.results]
    return np.ascontiguousarray(np.concatenate(shards, axis=1))


# revision 72
# speedup vs baseline: 1.0079x; 1.0007x over previous
"""Causal depthwise conv1d with learnable hidden-state prefix, on 8 TRN2 cores.

Reference computation (per batch b, channel d):
    xp = concat([init_state[d, :3], x[b, d, :]])          # [L+3] = [4099]
    out[b, d, t] = bias[d] + sum_{j=0..3} w[d, j] * xp[t+j]   for t in [0, 4099)
    (xp index beyond 4098 contributes 0)

Sharding: channel dim D=4096 split 8 ways (512 channels/core), zero
communication. Each core processes rows (b, d_local) = 4*512 = 2048 rows of
length 4096 -> 16 SBUF tiles of [128 rows, full row].

The kernel is DMA-bound: fp32 input (16.4KB/row) + bf16 output (8.2KB/row)
= ~140us at the 360GB/s descriptor-model roofline. Storing the output as
bf16 only rounds the final fp32 result (<= 2^-9 relative, inside the 2e-2
tolerance); the host upcasts back. All arithmetic stays fp32 (fp32r is an
11-bit-mantissa format - measured 2^-12 operand rounding violates the
near-zero-output error budget, so it is not used).

To fit every engine under the DMA roofline the 4-tap conv is spread over
all five engines per tile:
  - PE: first `pe_chunks` 512-col chunks as 4 accumulating diagonal-weight
    fp32 matmuls per PSUM bank; ACT evacuates + bias -> bf16.
  - remaining cols via a MAC chain in sub-pieces (pipelined): ACT computes
    A = w0*x0 + bias and B = w2*x2 (fp32 tmps), Pool merges A += B (plain
    TensorTensor Add - the only elementwise shape Pool's ISA accepts; STT
    with a per-partition scalar pointer is PE/ACT/DVE-only), DVE runs
    A += w1*x1 and the final out_bf16 = w3*x3 + A.  3 zero cols are padded
    after x so every tap runs full width (xp reads past the end get 0).
Out-DMA pieces are deferred a few tiles (banking transfer work for the
drain phase) and ride rings cross-engine from their producers: chain
pieces on the ACT HWDGE ring (DVE produces them), PE pieces on Pool
SWDGE, and everything on the idle SP ring during the drain. The last two
tiles run a third PE chunk (the PE is otherwise done) to shorten the
final chain latency.
"""

import numpy as np

B, D, L = 4, 4096, 4096
KTAPS = 4
K = KTAPS - 1          # 3: state length
LOUT = L + K           # 4099
NCORES = 8
DSH = D // NCORES      # 512 channels per core
ROWS = B * DSH         # 2048 rows per core
P = 128                # SBUF partitions
NTILES = ROWS // P     # 16
G = DSH // P           # 4 channel groups per core
MMCOLS = 512           # one PSUM bank of fp32 per matmul
WIN = 1 + K + L + K    # in_t cols: align pad, state, x, zero pad = 4103

_CACHE = {}


def _build_program(in_bufs=6, out_bufs=6, pe_chunks=(2,) * NTILES,
                   in_pieces=2, subs_m=3, subs_v=1, m_cols=3075,
                   chain_outs=1, out_lag=4, pe_tail=3, pe_tail_n=2,
                   ta_bufs=2, tb_bufs=2, pool_final=1):
    import concourse.bacc as bacc
    import concourse.mybir as mybir
    from concourse.tile import TileContext

    f32 = mybir.dt.float32
    bf16 = mybir.dt.bfloat16
    nc = bacc.Bacc("TRN2", target_bir_lowering=False, debug=False)

    xs = nc.dram_tensor("xs", [ROWS, L], f32, kind="ExternalInput").ap()
    # single packed param tensor -> single DMA -> single sync wait downstream.
    # layout per partition p: cols [g*4+j]=w[g*128+p, j] for g<4,j<4 (0..16),
    # col 16+g = bias[g*128+p], col 20+g*3+k = init_state[g*128+p, k]
    prm_d = nc.dram_tensor("prm", [P, 36], f32, kind="ExternalInput").ap()
    eye_d = nc.dram_tensor("eye", [P, P], f32, kind="ExternalInput").ap()
    out_d = nc.dram_tensor("out", [ROWS, LOUT], bf16, kind="ExternalOutput").ap()

    if pe_tail != 2:
        pe_chunks = (tuple(pe_chunks[:-pe_tail_n])
                     + (pe_tail,) * pe_tail_n)

    with TileContext(nc) as tc:
        with (
            tc.tile_pool(name="consts", bufs=1) as cpool,
            tc.tile_pool(name="xin", bufs=in_bufs) as in_pool,
            tc.tile_pool(name="yout", bufs=out_bufs) as out_pool,
            tc.tile_pool(name="tmpa", bufs=ta_bufs) as ta_pool,
            tc.tile_pool(name="tmpb", bufs=tb_bufs) as tb_pool,
            tc.tile_pool(name="psum", bufs=8, space="PSUM") as ps_pool,
        ):
            lpp = L // in_pieces

            def load_x(t):
                # in_t: col 0 pad (16B align), state [1:4), x [4:4100),
                # zero pad [4100:4103). x lands in pieces so chunks start
                # before the whole row arrives.
                in_t = in_pool.tile([P, WIN], f32)
                for p in range(in_pieces):
                    nc.sync.dma_start(
                        out=in_t[:, 1 + K + p * lpp:1 + K + (p + 1) * lpp],
                        in_=xs[t * P:(t + 1) * P, p * lpp:(p + 1) * lpp])
                return in_t

            # tile 0's x DMA first: it heads the SP/HWDGE pipeline, so its
            # transfer (the critical first 2.9us) isn't queued behind the
            # param loads, which have tiles of slack before first use.
            in_t0 = load_x(0)

            prm = cpool.tile([P, 36], f32)
            nc.sync.dma_start(out=prm, in_=prm_d)
            w_sb = prm[:, 0:G * KTAPS]
            b_sb = prm[:, 16:16 + G]
            s_sb = prm[:, 20:20 + G * K]
            zero = prm[:, 32:33]  # col 32: host leaves it zero (20:32 is state)

            # per-(group, tap) diagonal weight matrices for the PE path
            eye = cpool.tile([P, P], f32)
            nc.sync.dma_start(out=eye, in_=eye_d)
            dg = {}
            for g in range(G):
                for j in range(KTAPS):
                    d = cpool.tile([P, P], f32, tag=f"diag{g}_{j}")
                    nc.vector.tensor_scalar_mul(
                        out=d, in0=eye,
                        scalar1=w_sb[:, g * KTAPS + j:g * KTAPS + j + 1])
                    dg[(g, j)] = d

            lagged = []  # [tile, thunk] out-DMAs deferred by out_lag tiles

            for t in range(NTILES):
                g = t % G  # channel group (tile order: batch-major)
                rows = slice(t * P, (t + 1) * P)
                wj = [w_sb[:, g * KTAPS + j:g * KTAPS + j + 1]
                      for j in range(KTAPS)]

                in_t = in_t0 if t == 0 else load_x(t)
                nc.vector.tensor_scalar_mul(
                    out=in_t[:, 1:1 + K], in0=s_sb[:, g * K:(g + 1) * K],
                    scalar1=1.0)
                nc.vector.memset(in_t[:, 1 + K + L:], 0.0)

                out_t = out_pool.tile([P, LOUT], bf16)

                # flush out-DMAs deferred from >= out_lag tiles ago (all of
                # them on the last tile): their data is long done, so these
                # never park an engine SEQ mid-pipeline. The deferral banks
                # transfer work for the drain phase, when the remaining
                # compute can no longer saturate the DMA engines.
                flush_upto = t - out_lag
                lagged = [(tt, fn) for tt, fn in lagged
                          if not (tt <= flush_upto and (fn() or True))]

                # --- PE region: cols [0, ncols) ---
                # Evacs are emitted AFTER the chain taps (below): ACT's
                # in-order stream must not fence the PE-independent chain
                # work on the PE's PSUM completions.
                ncols = pe_chunks[t] * MMCOLS
                pss = []
                for c in range(pe_chunks[t]):
                    ps = ps_pool.tile([P, MMCOLS], f32)
                    base = 1 + c * MMCOLS
                    for j in range(KTAPS):
                        nc.tensor.matmul(
                            ps, dg[(g, j)],
                            in_t[:, base + j:base + j + MMCOLS],
                            start=(j == 0), stop=(j == KTAPS - 1))
                    pss.append(ps)

                # --- chain region: cols [ncols, LOUT) ---
                # merge flavor on [ncols, mend): ACT computes both chain
                # starts, Pool TT-adds them, DVE finishes (2 STTs).
                # DVE-serial flavor on the rest: ACT start + 3 DVE STTs.
                # Sized so ACT/DVE/Pool each stay under the DMA cadence.
                top = ncols
                mend = min(m_cols + ncols, LOUT)  # m boundary
                mtop = mend - top                 # m cols
                ub = [top + (mtop * s) // subs_m for s in range(subs_m + 1)]
                ub += [mend + ((LOUT - mend) * s) // subs_v
                       for s in range(1, subs_v + 1)]
                pairs = [(ub[s], ub[s + 1]) for s in range(len(ub) - 1)
                         if ub[s + 1] > ub[s]]
                for s, (lo, hi) in enumerate(pairs):
                    merge = hi <= mend
                    i0, i1 = 0, hi - lo
                    ta = ta_pool.tile([P, hi - lo], f32, tag=f"ta{s}")
                    if merge:
                        tb = tb_pool.tile([P, hi - lo], f32, tag=f"tb{s}")
                    nc.scalar.activation(
                        ta[:, i0:i1], in_t[:, 1 + lo:1 + hi],
                        mybir.ActivationFunctionType.Identity,
                        bias=b_sb[:, g:g + 1], scale=wj[0])
                    if merge and pool_final:
                        # 3-stage variant: two independent ACT+DVE
                        # sub-chains, Pool merges straight into bf16 out.
                        nc.scalar.activation(
                            tb[:, i0:i1], in_t[:, 3 + lo:3 + hi],
                            mybir.ActivationFunctionType.Identity,
                            bias=zero, scale=wj[2])
                        nc.vector.scalar_tensor_tensor(
                            out=ta[:, i0:i1], in0=in_t[:, 2 + lo:2 + hi],
                            scalar=wj[1], in1=ta[:, i0:i1],
                            op0=mybir.AluOpType.mult,
                            op1=mybir.AluOpType.add)
                        nc.vector.scalar_tensor_tensor(
                            out=tb[:, i0:i1], in0=in_t[:, 4 + lo:4 + hi],
                            scalar=wj[3], in1=tb[:, i0:i1],
                            op0=mybir.AluOpType.mult,
                            op1=mybir.AluOpType.add)
                        nc.gpsimd.tensor_tensor(
                            out=out_t[:, lo:hi], in0=ta[:, i0:i1],
                            in1=tb[:, i0:i1], op=mybir.AluOpType.add)
                        continue
                    if merge:
                        nc.scalar.activation(
                            tb[:, i0:i1], in_t[:, 3 + lo:3 + hi],
                            mybir.ActivationFunctionType.Identity,
                            bias=zero, scale=wj[2])
                        nc.gpsimd.tensor_tensor(
                            out=ta[:, i0:i1], in0=ta[:, i0:i1],
                            in1=tb[:, i0:i1], op=mybir.AluOpType.add)
                    else:
                        nc.vector.scalar_tensor_tensor(
                            out=ta[:, i0:i1],
                            in0=in_t[:, 3 + lo:3 + hi],
                            scalar=wj[2],
                            in1=ta[:, i0:i1],
                            op0=mybir.AluOpType.mult,
                            op1=mybir.AluOpType.add,
                        )
                    nc.vector.scalar_tensor_tensor(
                        out=ta[:, i0:i1],
                        in0=in_t[:, 2 + lo:2 + hi],
                        scalar=wj[1],
                        in1=ta[:, i0:i1],
                        op0=mybir.AluOpType.mult,
                        op1=mybir.AluOpType.add,
                    )
                    nc.vector.scalar_tensor_tensor(
                        out=out_t[:, lo:hi],
                        in0=in_t[:, 4 + lo:4 + hi],
                        scalar=wj[3],
                        in1=ta[:, i0:i1],
                        op0=mybir.AluOpType.mult,
                        op1=mybir.AluOpType.add,
                    )

                for c, ps in enumerate(pss):
                    nc.scalar.activation(
                        out_t[:, c * MMCOLS:(c + 1) * MMCOLS],
                        ps, mybir.ActivationFunctionType.Identity,
                        bias=b_sb[:, g:g + 1], scale=1.0)

                # --- out DMAs ---
                # PE piece on the SP ring (producer: ACT evac; lagging keeps
                # its wait pre-satisfied so it never head-of-line-blocks the
                # in stream); chain pieces on the ACT ring (producer: DVE
                # final STT) - cross-engine in both cases, so a real
                # semaphore guards the HW DMA against the producer pipeline.
                # All are lagged out_lag tiles: the banked transfers keep the
                # DMA engines fed during the drain phase, when the remaining
                # compute alone can't saturate them. The last tile ships one
                # piece per chain sub so the final transfer is small.
                def mk(ring, lo, hi, out_t=out_t, rows=rows):
                    def fire(last=False):
                        r = nc.sync if last else ring
                        r.dma_start(out=out_d[rows, lo:hi],
                                    in_=out_t[:, lo:hi])
                    return fire
                if t == NTILES - 1:
                    # drain: banked pieces ride the now-idle SP ring in tile
                    # order (oldest chains finish first), this tile's pieces
                    # follow per-sub on their usual rings. All engine compute
                    # is already dispatched, so SEQ parking is harmless.
                    for tt, fn in lagged:
                        fn(True)
                    lagged = []
                    mk(nc.sync, 0, ncols)()
                    for lo, hi in pairs:
                        mk(nc.scalar, lo, hi)()
                else:
                    thunks = [mk(nc.gpsimd, 0, ncols)]
                    cb = [ncols + ((LOUT - ncols) * s) // chain_outs
                          for s in range(chain_outs + 1)]
                    thunks += [mk(nc.scalar, cb[s], cb[s + 1])
                               for s in range(chain_outs)]
                    lagged += [(t, fn) for fn in thunks]

    nc.compile()
    return nc


def kernel(x, weight, bias, init_state):
    from concourse.bass_utils import run_bass_kernel_spmd

    assert x.shape == (B, D, L) and x.dtype == np.float32
    wl = np.ascontiguousarray(weight[:, 0, :], dtype=np.float32)      # [D, 4]
    bias = np.ascontiguousarray(bias, dtype=np.float32)               # [D]
    st = np.ascontiguousarray(init_state, dtype=np.float32)           # [D, 3]

    if "nc" not in _CACHE:
        _CACHE["nc"] = _build_program()
    nc = _CACHE["nc"]

    in_maps = []
    for c in range(NCORES):
        lo, hi = c * DSH, (c + 1) * DSH
        xs = np.ascontiguousarray(x[:, lo:hi, :]).reshape(ROWS, L)
        wc = wl[lo:hi]                                                # [512, 4]
        prm = np.zeros((P, 36), np.float32)
        prm[:, 0:G * KTAPS] = (
            wc.reshape(G, P, KTAPS).transpose(1, 0, 2).reshape(P, G * KTAPS))
        prm[:, 16:16 + G] = bias[lo:hi].reshape(G, P).T
        prm[:, 20:20 + G * K] = (
            st[lo:hi].reshape(G, P, K).transpose(1, 0, 2).reshape(P, G * K))
        in_maps.append({"xs": xs, "prm": prm,
                        "eye": np.eye(P, dtype=np.float32)})

    res = run_bass_kernel_spmd(nc, in_maps, core_ids=list(range(NCORES)))
    shards = [np.asarray(r["out"]).astype(np.float32).reshape(B, DSH, LOUT)
              for r in res.results]
    return np.ascontiguousarray(np.concatenate(shards, axis=1))
